# revision 43
# baseline (speedup 1.0000x reference)
"""Causal self-attention Trainium2 kernel (8 NeuronCores, SPMD).

Sharding: data-parallel over batch (B=2) x tensor-parallel over heads
(16 heads -> 4 per core).  core c: batch c//4, heads 4*(c%4) .. +4.
Each core computes qkv projection for its heads, causal attention, and a
partial out-projection; the host sums the 4 head-group partials per batch.

Layout notes:
  - Host passes x[b] pre-transposed (C, T) so the contraction dim C is
    partition-major for the qkv matmuls (PE contracts over partitions).
  - q,k are produced transposed (head_dim, T); v in natural (T, head_dim)
    with a ones column appended so the AV matmul also emits softmax row
    sums (row 64 of the PSUM accumulator).
  - scores are computed transposed (k, q) so the exp'd weights feed the
    AV matmul directly as the moving operand.
  - All matmul operands are bfloat16 (1 row/cycle at any moving width,
    half the DMA/SBUF/LDWEIGHTS traffic of fp32r).  PSUM stays fp32.
  - Causal masking: exp'd diagonal blocks are multiplied by a 0/1
    triangle mask on the (otherwise idle) GpSimd engine.
  - Softmax normalization: per-token reciprocal of the row sums via the
    fast approx-DVE reciprocal, then a K=1 ones-matmul broadcasts it
    across the 64 head partitions in PSUM, and one DVE multiply
    normalizes both heads of a pair at once.  No DRAM bounce.
"""

import numpy as np
import ml_dtypes

import concourse.bass as bass
import concourse.mybir as mybir
import concourse.tile as tile
from concourse import bacc
from concourse import bass_utils

# Problem shape (hardcoded per spec)
B, T, C = 2, 2048, 1024
NH, HD = 16, 64
NCORES = 8
HPC = 4                      # heads per core
P = 128                      # partitions
CB = C // P                  # 8 contraction blocks
QCW = 512                    # query chunk width
NQC = T // QCW               # 4 query chunks
NKB = T // P                 # 16 key blocks
SCALE = 1.0 / 8.0            # 1/sqrt(HD)

F32 = mybir.dt.float32
F32R = mybir.dt.float32r
BF16 = mybir.dt.bfloat16
EXP = mybir.ActivationFunctionType.Exp
NPBF16 = ml_dtypes.bfloat16


def build_program():
    nc = bacc.Bacc("TRN2", target_bir_lowering=False, debug=False,
                   num_devices=NCORES)

    xT = nc.dram_tensor("xT", [C, T], BF16, kind="ExternalInput").ap()
    wqk = nc.dram_tensor("wqk", [C, 2 * HPC * HD], BF16, kind="ExternalInput").ap()
    wv = nc.dram_tensor("wv", [C, HPC * HD], BF16, kind="ExternalInput").ap()
    wo = nc.dram_tensor("wo", [HPC * HD, C], BF16, kind="ExternalInput").ap()
    mask = nc.dram_tensor("mask", [P, P], BF16, kind="ExternalInput").ap()
    ones = nc.dram_tensor("ones", [P, HD], BF16, kind="ExternalInput").ap()
    sel = nc.dram_tensor("sel", [2, P], BF16, kind="ExternalInput").ap()
    y = nc.dram_tensor("y", [T, C], BF16, kind="ExternalOutput").ap()

    with tile.TileContext(nc) as tc:
        with tc.tile_pool(name="sb", bufs=1) as sb, \
             tc.tile_pool(name="work", bufs=1) as work, \
             tc.tile_pool(name="ps", bufs=1, space="PSUM") as ps:

            # ---- static loads, priority-tiered: the first qkv+attention
            # chunk only needs wqk and xT cols 0:512, so those go first
            wqk_sb = []
            wv_sb = []
            xT_sb = []
            xt_engs = [nc.scalar, nc.gpsimd]
            for cb in range(CB):
                t_ = sb.tile([P, 2 * HPC * HD], BF16, tag=f"wqk{cb}", bufs=1,
                             name=f"wqk_sb{cb}")
                nc.sync.dma_start(t_, wqk[cb * P:(cb + 1) * P, :])
                wqk_sb.append(t_)
                t_ = sb.tile([P, T], BF16, tag=f"xT{cb}", bufs=1,
                             name=f"xT_sb{cb}")
                xt_engs[cb % 2].dma_start(t_[:, 0:QCW],
                                          xT[cb * P:(cb + 1) * P, 0:QCW])
                xT_sb.append(t_)
            for cb in range(CB):
                t_ = sb.tile([P, HPC * HD], BF16, tag=f"wv{cb}", bufs=1,
                             name=f"wv_sb{cb}")
                nc.sync.dma_start(t_, wv[cb * P:(cb + 1) * P, :])
                wv_sb.append(t_)
            for tcg in range(1, NQC):
                csl = slice(tcg * QCW, (tcg + 1) * QCW)
                for cb in range(CB):
                    nc.scalar.dma_start(xT_sb[cb][:, csl],
                                        xT[cb * P:(cb + 1) * P, csl])
            wo_sb = []
            for hp in range(2):
                t_ = sb.tile([P, C], BF16, tag=f"wo{hp}", bufs=1,
                             name=f"wo_sb{hp}")
                nc.sync.dma_start(t_, wo[hp * P:(hp + 1) * P, :])
                wo_sb.append(t_)
            ones_sb = sb.tile([P, HD], BF16, tag="ones", bufs=1)
            nc.sync.dma_start(ones_sb, ones)
            mask_sb = sb.tile([P, P], BF16, tag="mask", bufs=1)
            nc.sync.dma_start(mask_sb, mask)
            sel_sb = sb.tile([2, P], BF16, tag="sel", bufs=1)
            nc.sync.dma_start(sel_sb, sel)
            # warm the exp table early (one tiny activation)
            exp_warm = sb.tile([1, HD], F32, tag="expwarm", bufs=1)
            nc.scalar.activation(exp_warm, ones_sb[0:1, :], EXP)

            # ---- qkv projection ----
            # qk transposed: qk_sb[jb] (128, T); jb 0,1 = q head pairs, 2,3 = k
            qk_sb = []
            for jb in range(4):
                t_ = sb.tile([P, T], BF16, tag=f"qk{jb}", bufs=1,
                             name=f"qk_sb{jb}")
                qk_sb.append(t_)
            # v natural per t-block, 4 heads x (64 v cols + ones col)
            v_sb = []
            for tb in range(NKB):
                t_ = sb.tile([P, HPC * (HD + 1)], BF16, tag=f"v{tb}", bufs=1,
                             name=f"v_sb{tb}")
                v_sb.append(t_)

            def qkv_units(tcg):
                """Yield PE-filler closures: one per qk psum group or v group."""
                tsl = slice(tcg * QCW, (tcg + 1) * QCW)

                def qk_unit(jb):
                    def emit():
                        ps_qk = ps.tile([P, QCW], F32, tag="ps", bufs=4,
                                        name="ps_qk")
                        for cb in range(CB):
                            nc.tensor.matmul(
                                ps_qk,
                                wqk_sb[cb][:, jb * P:(jb + 1) * P],
                                xT_sb[cb][:, tsl],
                                start=(cb == 0), stop=(cb == CB - 1))
                        nc.vector.tensor_copy(qk_sb[jb][:, tsl], ps_qk)
                    return emit

                def v_unit(tbl):
                    def emit():
                        tb = tcg * 4 + tbl
                        ps_v = ps.tile([P, HPC * HD], F32, tag="ps", bufs=4,
                                       name="ps_v")
                        for cb in range(CB):
                            nc.tensor.matmul(
                                ps_v,
                                xT_sb[cb][:, tb * P:(tb + 1) * P],
                                wv_sb[cb],
                                start=(cb == 0), stop=(cb == CB - 1))
                        vg = v_sb[tb].rearrange("p (h e) -> p h e", e=HD + 1)
                        nc.vector.tensor_copy(
                            vg[:, :, 0:HD],
                            ps_v.rearrange("p (h e) -> p h e", e=HD))
                        nc.vector.tensor_copy(
                            vg[:, :, HD:HD + 1],
                            ones_sb[:, 0:HPC].rearrange("p (h o) -> p h o", o=1))
                    return emit

                return [qk_unit(jb) for jb in range(4)] + \
                       [v_unit(tbl) for tbl in range(4)]

            def outproj_units(qc, attn, tail=False):
                # tail variant: per-half DMAs and copy/DMA spread across
                # three engines so the post-PE drain overlaps and shortens
                # y DMA issue stays off the Scalar queue: each DMA_DIRECT2D
                # occupies its engine ~600ns and the Scalar queue feeds the
                # latency-critical exp chain
                cp_engs = ([nc.vector, nc.scalar]
                           if tail else [nc.scalar, nc.vector])
                dma_engs = [nc.sync, nc.gpsimd]

                def op_unit(tbl):
                    def emit():
                        tb = qc * 4 + tbl
                        out_sb = work.tile([P, C], BF16, tag="outsb", bufs=4,
                                           name="out_sb")
                        for cob in range(2):
                            ps_o = ps.tile([P, QCW], F32, tag="ps", bufs=4,
                                           name="ps_o")
                            for hp in range(2):
                                nc.tensor.matmul(
                                    ps_o,
                                    attn[hp][:, tbl * P:(tbl + 1) * P],
                                    wo_sb[hp][:, cob * QCW:(cob + 1) * QCW],
                                    start=(hp == 0), stop=(hp == 1))
                            osl = out_sb[:, cob * QCW:(cob + 1) * QCW]
                            ceng = cp_engs[(2 * tbl + cob) % len(cp_engs)]
                            if ceng is nc.scalar:
                                ceng.copy(osl, ps_o)
                            else:
                                ceng.tensor_copy(osl, ps_o)
                            if tail:
                                deng = dma_engs[(2 * tbl + cob) % 2]
                                deng.dma_start(
                                    y[tb * P:(tb + 1) * P,
                                      cob * QCW:(cob + 1) * QCW], osl)
                        if not tail:
                            oeng = dma_engs[tbl % 2]
                            oeng.dma_start(y[tb * P:(tb + 1) * P, :], out_sb)
                    return emit
                return [op_unit(tbl) for tbl in range(4)]

            def emit_scores(qc, hp, kb):
                kofs = kb - qc * 4
                jofs = max(kofs, 0) * P
                w = QCW - jofs
                ps_s2 = ps.tile([P, 2 * QCW], F32, tag="ps2", bufs=2,
                                name="ps_s2")
                wexp2 = work.tile([P, 2 * QCW], BF16, tag="wexp2",
                                  bufs=6, name="wexp2")
                for par in range(2):
                    po = par * HD
                    nc.tensor.matmul(
                        ps_s2[:, par * QCW:par * QCW + w],
                        qk_sb[2 + hp][po:po + HD, kb * P:(kb + 1) * P],
                        qk_sb[hp][po:po + HD,
                                  qc * QCW + jofs:(qc + 1) * QCW],
                        start=True, stop=True)
                sview = ps_s2.rearrange("p (g q) -> p g q", g=2)
                wview = wexp2.rearrange("p (g q) -> p g q", g=2)
                nc.scalar.activation(wview[:, :, 0:w],
                                     sview[:, :, 0:w], EXP, scale=SCALE)
                if kofs >= 0:
                    # zero the masked triangle; one par on DVE, one on
                    # GpSimd so the two run in parallel and the
                    # exp->mask->AV chain is half as long
                    nc.vector.tensor_mul(wexp2[:, 0:P], wexp2[:, 0:P],
                                         mask_sb)
                    nc.gpsimd.tensor_mul(wexp2[:, QCW:QCW + P],
                                         wexp2[:, QCW:QCW + P], mask_sb)
                return (kb, wexp2, jofs, w)

            def emit_attention(qc, filler, tail_mk=None, pre=()):
                """scores -> fused exp -> AV, with PE filler interleaved to
                keep TensorE dense (HAM warm) while ACT chews the exps.

                Normalization is two-staged: stage A (no PE work: AV
                staging copies, sum copies, approx reciprocal, bf16 cast)
                is emitted inline; stage B (the K=1 broadcast matmul +
                final multiply) is deferred into the filler stream so the
                in-order PE queue never stalls on the cross-engine recip
                chain.  hp0's B runs as filler during hp1; hp1's B is
                returned and runs as filler in the next query chunk.  On
                the last chunk, B is fused per-128-column with the final
                out-projection units to pipeline the drain."""
                attn = {}
                nkb = (qc + 1) * 4
                n_units = 2 * nkb
                fq = list(filler)
                held = []
                if tail_mk is not None and len(fq) > 5:
                    # reserve filler to cover the tail's recip chain
                    held = fq[-5:]
                    fq = fq[:-5]
                rate = len(fq) / n_units if n_units else 0.0
                credit = 0.0
                deferred_b = None
                next_pre = []        # next-qc hp0 score blocks emitted early

                carry = list(pre)    # next-hp score blocks emitted early
                for hp in range(2):
                    at = work.tile([P, QCW], BF16, tag="attn", bufs=6,
                                   name=f"attn_hp{hp}")
                    attn[hp] = at
                    ps_av = {}
                    for par in range(2):
                        ps_av[par] = ps.tile([P, QCW], F32, tag="ps", bufs=4,
                                             name="ps_av")
                    pend = carry     # (kb, wexp2, jofs, w) awaiting AV
                    carry = []
                    for kb in range(len(pend), nkb):
                        pend.append(emit_scores(qc, hp, kb))
                        if len(pend) > 3:   # AV lags scores by 3 kb
                            _emit_av(hp, ps_av, pend.pop(0), nkb)
                        # drain PE filler to keep TensorE busy during exp
                        credit += rate
                        while credit >= 1.0 and fq:
                            fq.pop(0)()
                            credit -= 1.0
                    if hp == 0:
                        # software-pipeline across the hp boundary: the
                        # next hp's first score blocks are independent PE
                        # work, and ACT starts their exps while this hp's
                        # AVs drain -- no refill bubble at the transition
                        for pre_kb in range(2):
                            carry.append(emit_scores(qc, 1, pre_kb))
                            if pend:
                                _emit_av(hp, ps_av, pend.pop(0), nkb)
                    elif tail_mk is None:
                        # same pipelining across the qc boundary.  The
                        # filler must drain first: it contains the next
                        # chunk's qkv units, which the prefetched scores
                        # depend on, and the PE queue is in-order.
                        while fq:
                            fq.pop(0)()
                        for pre_kb in range(2):
                            next_pre.append(emit_scores(qc + 1, 0, pre_kb))
                            if pend:
                                _emit_av(hp, ps_av, pend.pop(0), nkb)
                    while pend:
                        _emit_av(hp, ps_av, pend.pop(0), nkb)
                        if fq:
                            fq.pop(0)()
                    # --- norm stage A (no PE instructions) ---
                    # stage AV out of PSUM promptly so the banks recycle;
                    # heads land at their attn-aligned partition offsets.
                    # The recip chain leads the DVE queue so the broadcast
                    # matmul unblocks as early as possible; at the tail the
                    # exp stream is finished, so ACT takes both av_st
                    # halves and the DVE chain shortens further.
                    av_st = work.tile([P, QCW], BF16, tag="avst",
                                      bufs=4, name="av_st")
                    is_tail = tail_mk is not None and hp == 1
                    rp2s = []
                    for par in range(2):
                        sp = work.tile([1, QCW], F32, tag=f"sum{par}", bufs=3,
                                       name=f"sum_p{par}")
                        nc.vector.tensor_copy(sp, ps_av[par][HD:HD + 1, :])
                        rp = work.tile([1, QCW], F32, tag=f"rec{par}", bufs=3,
                                       name=f"rec_p{par}")
                        nc.vector.reciprocal_approx_fast(rp, sp)
                        rp2 = work.tile([1, QCW], BF16, tag=f"recb{par}",
                                        bufs=3, name=f"recb_p{par}")
                        nc.vector.tensor_copy(rp2, rp)
                        rp2s.append(rp2)
                    nc.scalar.copy(av_st[HD:P, :], ps_av[1][0:HD, :])
                    if is_tail:
                        nc.scalar.copy(av_st[0:HD, :], ps_av[0][0:HD, :])
                    else:
                        nc.vector.tensor_copy(av_st[0:HD, :],
                                              ps_av[0][0:HD, :])

                    def mk_b(at=at, av_st=av_st, rp2s=rp2s):
                        def b_unit():
                            ps_b = ps.tile([P, QCW], F32, tag="ps", bufs=4,
                                           name="ps_b")
                            for par in range(2):
                                nc.tensor.matmul(
                                    ps_b[par * HD:(par + 1) * HD, :],
                                    sel_sb[0:1, 0:HD], rp2s[par],
                                    start=True, stop=True)
                            nc.vector.tensor_mul(at, av_st, ps_b)
                        return b_unit

                    if hp == 0:
                        # a couple of units deep so the PE does not stall
                        # on the still-in-flight recip chain
                        fq.insert(min(2, len(fq)), mk_b())
                    elif tail_mk is None:
                        deferred_b = mk_b()
                    else:
                        # last chunk: drain remaining + held filler first
                        # (independent PE work covering the recip chain),
                        # then fuse broadcast+mul+outproj per 128-col chunk
                        while fq:
                            fq.pop(0)()
                        for u in held:
                            u()
                        tail_units = tail_mk(attn)
                        # ps_b from the ps2 ring: the exp pipeline is done,
                        # so no WAR chain through in-flight ps_o/out_sb/DMA
                        ps_b2 = ps.tile([P, 2 * QCW], F32, tag="ps2", bufs=2,
                                        name="ps_b2")
                        ps_b = ps_b2[:, 0:QCW]
                        for tbl in range(4):
                            csl = slice(tbl * P, (tbl + 1) * P)
                            for par in range(2):
                                nc.tensor.matmul(
                                    ps_b[par * HD:(par + 1) * HD, csl],
                                    sel_sb[0:1, 0:HD], rp2s[par][:, csl],
                                    start=True, stop=True)
                            nc.vector.tensor_mul(at[:, csl], av_st[:, csl],
                                                 ps_b[:, csl])
                            tail_units[tbl]()
                while fq:
                    fq.pop(0)()
                return attn, deferred_b, next_pre

            def _emit_av(hp, ps_av, pend, nkb):
                kb, wexp2, jofs, w = pend
                for par in range(2):
                    h = 2 * hp + par
                    nc.tensor.matmul(
                        ps_av[par][0:HD + 1, jofs:QCW],
                        v_sb[kb][:, h * (HD + 1):(h + 1) * (HD + 1)],
                        wexp2[:, par * QCW:par * QCW + w],
                        start=(kb == 0), stop=(kb == nkb - 1))

            # emission schedule: qkv(0) plain; attention(t) with qkv(t+1)
            # and outproj(t-1) interleaved as PE filler.
            for u in qkv_units(0):
                u()
            # qkv(1) qk-units inline: attention(1) must not wait on them
            for u in qkv_units(1)[:4]:
                u()
            attns = []
            prev_b = None
            pre = ()
            for tcg in range(NQC):
                if tcg == 0:
                    filler = qkv_units(1)[4:]
                elif tcg == 1:
                    filler = qkv_units(2) + outproj_units(0, attns[0])
                elif tcg == 2:
                    filler = qkv_units(3)
                else:
                    filler = (outproj_units(1, attns[1])
                              + outproj_units(2, attns[2]))
                if prev_b is not None:
                    filler = filler[:2] + [prev_b] + filler[2:]
                tail_mk = None
                if tcg == NQC - 1:
                    tail_mk = lambda attn: outproj_units(NQC - 1, attn,
                                                         tail=True)
                attn, prev_b, pre = emit_attention(tcg, filler, tail_mk, pre)
                attns.append(attn)

    nc.compile()
    return nc


_PROGRAM = None


def _get_program():
    global _PROGRAM
    if _PROGRAM is None:
        _PROGRAM = build_program()
    return _PROGRAM


def make_in_maps(x, w_qkv, w_out):
    mask = np.triu(np.ones((P, P), dtype=np.float32))  # keep k<=q: i<=j
    selm = np.zeros((2, P), dtype=np.float32)
    selm[0, 0:HD] = 1.0
    selm[1, HD:2 * HD] = 1.0
    in_maps = []
    for core in range(NCORES):
        b, p = core // HPC, core % HPC
        h0 = p * HPC * HD                       # first head col offset (256*p)
        in_maps.append({
            "xT": np.ascontiguousarray(x[b].T).astype(NPBF16),
            "wqk": np.ascontiguousarray(np.concatenate(
                [w_qkv[:, h0:h0 + HPC * HD],
                 w_qkv[:, C + h0:C + h0 + HPC * HD]], axis=1)).astype(NPBF16),
            "wv": np.ascontiguousarray(
                w_qkv[:, 2 * C + h0:2 * C + h0 + HPC * HD]).astype(NPBF16),
            "wo": np.ascontiguousarray(w_out[h0:h0 + HPC * HD, :]).astype(NPBF16),
            "mask": mask.astype(NPBF16),
            "ones": np.ones((P, HD), dtype=NPBF16),
            "sel": selm.astype(NPBF16),
        })
    return in_maps


def kernel(x, w_qkv, w_out):
    x = np.asarray(x, dtype=np.float32)
    w_qkv = np.asarray(w_qkv, dtype=np.float32)
    w_out = np.asarray(w_out, dtype=np.float32)
    nc = _get_program()
    res = bass_utils.run_bass_kernel_spmd(nc, make_in_maps(x, w_qkv, w_out),
                                          core_ids=list(range(NCORES)))
    y = np.zeros((B, T, C), dtype=np.float32)
    for core in range(NCORES):
        y[core // HPC] += res.results[core]["y"].astype(np.float32)
    return y


# revision 45
# speedup vs baseline: 1.1658x; 1.1658x over previous
"""Causal self-attention Trainium2 kernel (8 NeuronCores, SPMD).

Sharding: data-parallel over batch (B=2) x tensor-parallel over heads
(16 heads -> 4 per core).  core c: batch c//4, heads 4*(c%4) .. +4.
Each core computes qkv projection for its heads, causal attention, and a
partial out-projection; the host sums the 4 head-group partials per batch.

Layout notes:
  - Host passes x[b] pre-transposed (C, T) so the contraction dim C is
    partition-major for the qkv matmuls (PE contracts over partitions).
  - q,k are produced transposed (head_dim, T); v in natural (T, head_dim)
    with a ones column appended so the AV matmul also emits softmax row
    sums (row 64 of the PSUM accumulator).
  - scores are computed transposed (k, q) so the exp'd weights feed the
    AV matmul directly as the moving operand.
  - All matmul operands are bfloat16 (1 row/cycle at any moving width,
    half the DMA/SBUF/LDWEIGHTS traffic of fp32r).  PSUM stays fp32.
  - Causal masking: exp'd diagonal blocks are multiplied by a 0/1
    triangle mask, one head-par on DVE and one on GpSimd in parallel.
  - Softmax normalization: per-token reciprocal of the row sums via the
    fast approx-DVE reciprocal, then a K=1 ones-matmul broadcasts it
    across the 64 head partitions in PSUM, and one DVE multiply
    normalizes both heads of a pair at once.  No DRAM bounce.
  - Scheduling: the in-order PE queue never waits on a cross-engine
    chain -- normalization stage B is deferred into the filler stream,
    score blocks are prefetched across hp/qc boundaries, and the final
    out-projection is fused per-128-column with the last chunk's
    normalization to pipeline the drain.
"""

import numpy as np
import ml_dtypes

import concourse.bass as bass
import concourse.mybir as mybir
import concourse.tile as tile
from concourse import bacc
from concourse import bass_utils

# Problem shape (hardcoded per spec)
B, T, C = 2, 2048, 1024
NH, HD = 16, 64
NCORES = 8
HPC = 4                      # heads per core
P = 128                      # partitions
CB = C // P                  # 8 contraction blocks
QCW = 512                    # query chunk width
NQC = T // QCW               # 4 query chunks
NKB = T // P                 # 16 key blocks
SCALE = 1.0 / 8.0            # 1/sqrt(HD)

F32 = mybir.dt.float32
BF16 = mybir.dt.bfloat16
EXP = mybir.ActivationFunctionType.Exp
NPBF16 = ml_dtypes.bfloat16


def build_program():
    nc = bacc.Bacc("TRN2", target_bir_lowering=False, debug=False,
                   num_devices=NCORES)

    xT = nc.dram_tensor("xT", [C, T], BF16, kind="ExternalInput").ap()
    wqk = nc.dram_tensor("wqk", [C, 2 * HPC * HD], BF16, kind="ExternalInput").ap()
    wv = nc.dram_tensor("wv", [C, HPC * HD], BF16, kind="ExternalInput").ap()
    wo = nc.dram_tensor("wo", [HPC * HD, C], BF16, kind="ExternalInput").ap()
    mask = nc.dram_tensor("mask", [P, P], BF16, kind="ExternalInput").ap()
    ones = nc.dram_tensor("ones", [P, HD], BF16, kind="ExternalInput").ap()
    sel = nc.dram_tensor("sel", [2, P], BF16, kind="ExternalInput").ap()
    y = nc.dram_tensor("y", [T, C], BF16, kind="ExternalOutput").ap()

    with tile.TileContext(nc) as tc:
        with tc.tile_pool(name="sb", bufs=1) as sb, \
             tc.tile_pool(name="work", bufs=1) as work, \
             tc.tile_pool(name="ps", bufs=1, space="PSUM") as ps:

            # ---- static loads, priority-tiered: the first qkv+attention
            # chunk only needs wqk and xT cols 0:512, so those go first
            wqk_sb = []
            wv_sb = []
            xT_sb = []
            xt_engs = [nc.scalar, nc.gpsimd]
            for cb in range(CB):
                t_ = sb.tile([P, 2 * HPC * HD], BF16, tag=f"wqk{cb}", bufs=1,
                             name=f"wqk_sb{cb}")
                nc.sync.dma_start(t_, wqk[cb * P:(cb + 1) * P, :])
                wqk_sb.append(t_)
                t_ = sb.tile([P, T], BF16, tag=f"xT{cb}", bufs=1,
                             name=f"xT_sb{cb}")
                xt_engs[cb % 2].dma_start(t_[:, 0:QCW],
                                          xT[cb * P:(cb + 1) * P, 0:QCW])
                xT_sb.append(t_)
            for cb in range(CB):
                t_ = sb.tile([P, HPC * HD], BF16, tag=f"wv{cb}", bufs=1,
                             name=f"wv_sb{cb}")
                nc.sync.dma_start(t_, wv[cb * P:(cb + 1) * P, :])
                wv_sb.append(t_)
            for tcg in range(1, NQC):
                csl = slice(tcg * QCW, (tcg + 1) * QCW)
                for cb in range(CB):
                    nc.scalar.dma_start(xT_sb[cb][:, csl],
                                        xT[cb * P:(cb + 1) * P, csl])
            wo_sb = []
            for hp in range(2):
                t_ = sb.tile([P, C], BF16, tag=f"wo{hp}", bufs=1,
                             name=f"wo_sb{hp}")
                nc.sync.dma_start(t_, wo[hp * P:(hp + 1) * P, :])
                wo_sb.append(t_)
            ones_sb = sb.tile([P, HD], BF16, tag="ones", bufs=1)
            nc.sync.dma_start(ones_sb, ones)
            mask_sb = sb.tile([P, P], BF16, tag="mask", bufs=1)
            nc.sync.dma_start(mask_sb, mask)
            sel_sb = sb.tile([2, P], BF16, tag="sel", bufs=1)
            nc.sync.dma_start(sel_sb, sel)
            # warm the exp table early (one tiny activation)
            exp_warm = sb.tile([1, HD], F32, tag="expwarm", bufs=1)
            nc.scalar.activation(exp_warm, ones_sb[0:1, :], EXP)

            # ---- qkv projection ----
            # qk transposed: qk_sb[jb] (128, T); jb 0,1 = q head pairs, 2,3 = k
            qk_sb = []
            for jb in range(4):
                t_ = sb.tile([P, T], BF16, tag=f"qk{jb}", bufs=1,
                             name=f"qk_sb{jb}")
                qk_sb.append(t_)
            # v natural per t-block, 4 heads x (64 v cols + ones col)
            v_sb = []
            for tb in range(NKB):
                t_ = sb.tile([P, HPC * (HD + 1)], BF16, tag=f"v{tb}", bufs=1,
                             name=f"v_sb{tb}")
                v_sb.append(t_)

            def qkv_units(tcg):
                """Yield PE-filler closures: one per qk psum group or v group."""
                tsl = slice(tcg * QCW, (tcg + 1) * QCW)

                def qk_unit(jb):
                    def emit():
                        ps_qk = ps.tile([P, QCW], F32, tag="ps", bufs=4,
                                        name="ps_qk")
                        for cb in range(CB):
                            nc.tensor.matmul(
                                ps_qk,
                                wqk_sb[cb][:, jb * P:(jb + 1) * P],
                                xT_sb[cb][:, tsl],
                                start=(cb == 0), stop=(cb == CB - 1))
                        nc.vector.tensor_copy(qk_sb[jb][:, tsl], ps_qk)
                    return emit

                def v_unit(tbl):
                    def emit():
                        tb = tcg * 4 + tbl
                        ps_v = ps.tile([P, HPC * HD], F32, tag="ps", bufs=4,
                                       name="ps_v")
                        for cb in range(CB):
                            nc.tensor.matmul(
                                ps_v,
                                xT_sb[cb][:, tb * P:(tb + 1) * P],
                                wv_sb[cb],
                                start=(cb == 0), stop=(cb == CB - 1))
                        vg = v_sb[tb].rearrange("p (h e) -> p h e", e=HD + 1)
                        nc.vector.tensor_copy(
                            vg[:, :, 0:HD],
                            ps_v.rearrange("p (h e) -> p h e", e=HD))
                        nc.vector.tensor_copy(
                            vg[:, :, HD:HD + 1],
                            ones_sb[:, 0:HPC].rearrange("p (h o) -> p h o", o=1))
                    return emit

                return [qk_unit(jb) for jb in range(4)] + \
                       [v_unit(tbl) for tbl in range(4)]

            def outproj_units(qc, attn, tail=False):
                # tail variant: per-half DMAs and copy/DMA spread across
                # three engines so the post-PE drain overlaps and shortens
                # y DMA issue stays off the Scalar queue: each DMA_DIRECT2D
                # occupies its engine ~600ns and the Scalar queue feeds the
                # latency-critical exp chain
                cp_engs = ([nc.vector, nc.scalar]
                           if tail else [nc.scalar, nc.vector])
                dma_engs = [nc.sync, nc.gpsimd]

                def op_unit(tbl):
                    def emit():
                        tb = qc * 4 + tbl
                        out_sb = work.tile([P, C], BF16, tag="outsb", bufs=4,
                                           name="out_sb")
                        for cob in range(2):
                            ps_o = ps.tile([P, QCW], F32, tag="ps", bufs=4,
                                           name="ps_o")
                            for hp in range(2):
                                nc.tensor.matmul(
                                    ps_o,
                                    attn[hp][:, tbl * P:(tbl + 1) * P],
                                    wo_sb[hp][:, cob * QCW:(cob + 1) * QCW],
                                    start=(hp == 0), stop=(hp == 1))
                            osl = out_sb[:, cob * QCW:(cob + 1) * QCW]
                            ceng = cp_engs[(2 * tbl + cob) % len(cp_engs)]
                            if ceng is nc.scalar:
                                ceng.copy(osl, ps_o)
                            else:
                                ceng.tensor_copy(osl, ps_o)
                            if tail:
                                deng = dma_engs[(2 * tbl + cob) % 2]
                                deng.dma_start(
                                    y[tb * P:(tb + 1) * P,
                                      cob * QCW:(cob + 1) * QCW], osl)
                        if not tail:
                            oeng = dma_engs[tbl % 2]
                            oeng.dma_start(y[tb * P:(tb + 1) * P, :], out_sb)
                    return emit
                return [op_unit(tbl) for tbl in range(4)]

            def emit_scores(qc, hp, kb):
                kofs = kb - qc * 4
                jofs = max(kofs, 0) * P
                w = QCW - jofs
                ps_s2 = ps.tile([P, 2 * QCW], F32, tag="ps2", bufs=2,
                                name="ps_s2")
                wexp2 = work.tile([P, 2 * QCW], BF16, tag="wexp2",
                                  bufs=6, name="wexp2")
                for par in range(2):
                    po = par * HD
                    nc.tensor.matmul(
                        ps_s2[:, par * QCW:par * QCW + w],
                        qk_sb[2 + hp][po:po + HD, kb * P:(kb + 1) * P],
                        qk_sb[hp][po:po + HD,
                                  qc * QCW + jofs:(qc + 1) * QCW],
                        start=True, stop=True)
                sview = ps_s2.rearrange("p (g q) -> p g q", g=2)
                wview = wexp2.rearrange("p (g q) -> p g q", g=2)
                nc.scalar.activation(wview[:, :, 0:w],
                                     sview[:, :, 0:w], EXP, scale=SCALE)
                if kofs >= 0:
                    # zero the masked triangle; one par on DVE, one on
                    # GpSimd so the two run in parallel and the
                    # exp->mask->AV chain is half as long
                    nc.vector.tensor_mul(wexp2[:, 0:P], wexp2[:, 0:P],
                                         mask_sb)
                    nc.gpsimd.tensor_mul(wexp2[:, QCW:QCW + P],
                                         wexp2[:, QCW:QCW + P], mask_sb)
                return (kb, wexp2, jofs, w)

            def emit_attention(qc, filler, tail_mk=None, pre=()):
                """scores -> fused exp -> AV, with PE filler interleaved to
                keep TensorE dense (HAM warm) while ACT chews the exps.

                Normalization is two-staged: stage A (no PE work: AV
                staging copies, sum copies, approx reciprocal, bf16 cast)
                is emitted inline; stage B (the K=1 broadcast matmul +
                final multiply) is deferred into the filler stream so the
                in-order PE queue never stalls on the cross-engine recip
                chain.  hp0's B runs as filler during hp1; hp1's B is
                returned and runs as filler in the next query chunk.  On
                the last chunk, B is fused per-128-column with the final
                out-projection units to pipeline the drain."""
                attn = {}
                nkb = (qc + 1) * 4
                n_units = 2 * nkb
                fq = list(filler)
                held = []
                if tail_mk is not None and len(fq) > 5:
                    # reserve filler to cover the tail's recip chain
                    held = fq[-5:]
                    fq = fq[:-5]
                rate = len(fq) / n_units if n_units else 0.0
                credit = 0.0
                deferred_b = None
                next_pre = []        # next-qc hp0 score blocks emitted early

                carry = list(pre)    # next-hp score blocks emitted early
                for hp in range(2):
                    at = work.tile([P, QCW], BF16, tag="attn", bufs=6,
                                   name=f"attn_hp{hp}")
                    attn[hp] = at
                    ps_av = {}
                    for par in range(2):
                        ps_av[par] = ps.tile([P, QCW], F32, tag="ps", bufs=4,
                                             name="ps_av")
                    pend = carry     # (kb, wexp2, jofs, w) awaiting AV
                    carry = []
                    for kb in range(len(pend), nkb):
                        pend.append(emit_scores(qc, hp, kb))
                        if len(pend) > 3:   # AV lags scores by 3 kb
                            _emit_av(hp, ps_av, pend.pop(0), nkb)
                        # drain PE filler to keep TensorE busy during exp
                        credit += rate
                        while credit >= 1.0 and fq:
                            fq.pop(0)()
                            credit -= 1.0
                    if hp == 0:
                        # software-pipeline across the hp boundary: the
                        # next hp's first score blocks are independent PE
                        # work, and ACT starts their exps while this hp's
                        # AVs drain -- no refill bubble at the transition
                        for pre_kb in range(2):
                            carry.append(emit_scores(qc, 1, pre_kb))
                            if pend:
                                _emit_av(hp, ps_av, pend.pop(0), nkb)
                    elif tail_mk is None:
                        # same pipelining across the qc boundary.  The
                        # filler must drain first: it contains the next
                        # chunk's qkv units, which the prefetched scores
                        # depend on, and the PE queue is in-order.
                        while fq:
                            fq.pop(0)()
                        for pre_kb in range(2):
                            next_pre.append(emit_scores(qc + 1, 0, pre_kb))
                            if pend:
                                _emit_av(hp, ps_av, pend.pop(0), nkb)
                    while pend:
                        _emit_av(hp, ps_av, pend.pop(0), nkb)
                        if fq:
                            fq.pop(0)()
                    # --- norm stage A (no PE instructions) ---
                    # stage AV out of PSUM promptly so the banks recycle;
                    # heads land at their attn-aligned partition offsets.
                    # The recip chain leads the DVE queue so the broadcast
                    # matmul unblocks as early as possible; at the tail the
                    # exp stream is finished, so ACT takes both av_st
                    # halves and the DVE chain shortens further.
                    av_st = work.tile([P, QCW], BF16, tag="avst",
                                      bufs=4, name="av_st")
                    is_tail = tail_mk is not None and hp == 1
                    rp2s = []
                    for par in range(2):
                        sp = work.tile([1, QCW], F32, tag=f"sum{par}", bufs=3,
                                       name=f"sum_p{par}")
                        nc.vector.tensor_copy(sp, ps_av[par][HD:HD + 1, :])
                        rp = work.tile([1, QCW], F32, tag=f"rec{par}", bufs=3,
                                       name=f"rec_p{par}")
                        nc.vector.reciprocal_approx_fast(rp, sp)
                        rp2 = work.tile([1, QCW], BF16, tag=f"recb{par}",
                                        bufs=3, name=f"recb_p{par}")
                        nc.vector.tensor_copy(rp2, rp)
                        rp2s.append(rp2)
                    nc.scalar.copy(av_st[HD:P, :], ps_av[1][0:HD, :])
                    if is_tail:
                        nc.scalar.copy(av_st[0:HD, :], ps_av[0][0:HD, :])
                    else:
                        nc.vector.tensor_copy(av_st[0:HD, :],
                                              ps_av[0][0:HD, :])

                    def mk_b(at=at, av_st=av_st, rp2s=rp2s):
                        def b_unit():
                            ps_b = ps.tile([P, QCW], F32, tag="ps", bufs=4,
                                           name="ps_b")
                            for par in range(2):
                                nc.tensor.matmul(
                                    ps_b[par * HD:(par + 1) * HD, :],
                                    sel_sb[0:1, 0:HD], rp2s[par],
                                    start=True, stop=True)
                            nc.vector.tensor_mul(at, av_st, ps_b)
                        return b_unit

                    if hp == 0:
                        # a couple of units deep so the PE does not stall
                        # on the still-in-flight recip chain
                        fq.insert(min(2, len(fq)), mk_b())
                    elif tail_mk is None:
                        deferred_b = mk_b()
                    else:
                        # last chunk: drain remaining + held filler first
                        # (independent PE work covering the recip chain),
                        # then fuse broadcast+mul+outproj per 128-col chunk
                        while fq:
                            fq.pop(0)()
                        for u in held:
                            u()
                        tail_units = tail_mk(attn)
                        # ps_b from the ps2 ring: the exp pipeline is done,
                        # so no WAR chain through in-flight ps_o/out_sb/DMA
                        ps_b2 = ps.tile([P, 2 * QCW], F32, tag="ps2", bufs=2,
                                        name="ps_b2")
                        ps_b = ps_b2[:, 0:QCW]
                        for tbl in range(4):
                            csl = slice(tbl * P, (tbl + 1) * P)
                            for par in range(2):
                                nc.tensor.matmul(
                                    ps_b[par * HD:(par + 1) * HD, csl],
                                    sel_sb[0:1, 0:HD], rp2s[par][:, csl],
                                    start=True, stop=True)
                            nc.vector.tensor_mul(at[:, csl], av_st[:, csl],
                                                 ps_b[:, csl])
                            tail_units[tbl]()
                while fq:
                    fq.pop(0)()
                return attn, deferred_b, next_pre

            def _emit_av(hp, ps_av, pend, nkb):
                kb, wexp2, jofs, w = pend
                for par in range(2):
                    h = 2 * hp + par
                    nc.tensor.matmul(
                        ps_av[par][0:HD + 1, jofs:QCW],
                        v_sb[kb][:, h * (HD + 1):(h + 1) * (HD + 1)],
                        wexp2[:, par * QCW:par * QCW + w],
                        start=(kb == 0), stop=(kb == nkb - 1))

            # emission schedule: qkv(0) plain; attention(t) with qkv(t+1)
            # and outproj(t-1) interleaved as PE filler.
            for u in qkv_units(0):
                u()
            # qkv(1) qk-units inline: attention(1) must not wait on them
            for u in qkv_units(1)[:4]:
                u()
            attns = []
            prev_b = None
            pre = ()
            for tcg in range(NQC):
                if tcg == 0:
                    filler = qkv_units(1)[4:]
                elif tcg == 1:
                    filler = qkv_units(2) + outproj_units(0, attns[0])
                elif tcg == 2:
                    filler = qkv_units(3)
                else:
                    filler = (outproj_units(1, attns[1])
                              + outproj_units(2, attns[2]))
                if prev_b is not None:
                    filler = filler[:2] + [prev_b] + filler[2:]
                tail_mk = None
                if tcg == NQC - 1:
                    tail_mk = lambda attn: outproj_units(NQC - 1, attn,
                                                         tail=True)
                attn, prev_b, pre = emit_attention(tcg, filler, tail_mk, pre)
                attns.append(attn)

    nc.compile()
    return nc


_PROGRAM = None


def _get_program():
    global _PROGRAM
    if _PROGRAM is None:
        _PROGRAM = build_program()
    return _PROGRAM


def make_in_maps(x, w_qkv, w_out):
    mask = np.triu(np.ones((P, P), dtype=np.float32))  # keep k<=q: i<=j
    selm = np.zeros((2, P), dtype=np.float32)
    selm[0, 0:HD] = 1.0
    selm[1, HD:2 * HD] = 1.0
    in_maps = []
    for core in range(NCORES):
        b, p = core // HPC, core % HPC
        h0 = p * HPC * HD                       # first head col offset (256*p)
        in_maps.append({
            "xT": np.ascontiguousarray(x[b].T).astype(NPBF16),
            "wqk": np.ascontiguousarray(np.concatenate(
                [w_qkv[:, h0:h0 + HPC * HD],
                 w_qkv[:, C + h0:C + h0 + HPC * HD]], axis=1)).astype(NPBF16),
            "wv": np.ascontiguousarray(
                w_qkv[:, 2 * C + h0:2 * C + h0 + HPC * HD]).astype(NPBF16),
            "wo": np.ascontiguousarray(w_out[h0:h0 + HPC * HD, :]).astype(NPBF16),
            "mask": mask.astype(NPBF16),
            "ones": np.ones((P, HD), dtype=NPBF16),
            "sel": selm.astype(NPBF16),
        })
    return in_maps


def kernel(x, w_qkv, w_out):
    x = np.asarray(x, dtype=np.float32)
    w_qkv = np.asarray(w_qkv, dtype=np.float32)
    w_out = np.asarray(w_out, dtype=np.float32)
    nc = _get_program()
    res = bass_utils.run_bass_kernel_spmd(nc, make_in_maps(x, w_qkv, w_out),
                                          core_ids=list(range(NCORES)))
    y = np.zeros((B, T, C), dtype=np.float32)
    for core in range(NCORES):
        y[core // HPC] += res.results[core]["y"].astype(np.float32)
    return y
